# revision 10
# baseline (speedup 1.0000x reference)
"""Seq2seq RNN with attention on 8 TRN2 NeuronCores.

Strategy: pure data-parallel over batch. B=32 -> 4 batch elements per core.
Each core runs the full encoder (2-layer tanh RNN), decoder (tanh RNN +
dot-product attention) and the final vocab projection (d=256 -> V=32000)
for its batch shard. Host concatenates the per-core logits along batch.

On-device layout: hidden states kept transposed (d on partitions, batch on
free dim) so the recurrent matmul h@U becomes U.T-chunk matmuls with the
natural U layout as lhsT and no per-step transposes.
"""

import numpy as np

import concourse.bass as bass
import concourse.bacc as bacc
import concourse.tile as tile
from concourse import mybir
from concourse.bass_utils import run_bass_kernel_spmd
from concourse.masks import make_identity

D = 256
V = 32000
T = 128  # T_SRC == T_TGT == 128
B = 32
NCORES = 8
BL = B // NCORES  # 4 batch elements per core
KC = D // 128  # 2 d-chunks of 128
DT = mybir.dt.float32
AF = mybir.ActivationFunctionType
ALU = mybir.AluOpType
AX = mybir.AxisListType

_CACHE = {}
DEBUG = False


def _build():
    nc = bacc.Bacc(None)

    u_d = nc.declare_dram_parameter("u", [D, D], DT, isOutput=False)
    cwt_d = nc.declare_dram_parameter("ctx_wt", [D, D], DT, isOutput=False)
    wot_d = nc.declare_dram_parameter("w_out_t", [D, V], DT, isOutput=False)
    een_d = nc.declare_dram_parameter("e_en", [V, D], DT, isOutput=False)
    ede_d = nc.declare_dram_parameter("e_de", [V, D], DT, isOutput=False)
    b1_d = nc.declare_dram_parameter("b1", [128, KC], DT, isOutput=False)
    b2_d = nc.declare_dram_parameter("b2", [128, KC], DT, isOutput=False)
    bd_d = nc.declare_dram_parameter("bd", [128, KC], DT, isOutput=False)
    si_d = nc.declare_dram_parameter("src_idx", [T, BL], mybir.dt.int32, isOutput=False)
    ti_d = nc.declare_dram_parameter("tgt_idx", [T, BL], mybir.dt.int32, isOutput=False)
    out_d = nc.declare_dram_parameter("out", [T * BL, V], DT, isOutput=True)
    if DEBUG:
        dbg_xs = nc.declare_dram_parameter("dbg_xs", [128, KC * BL * T], DT, isOutput=True)
        dbg_hd = nc.declare_dram_parameter("dbg_hd", [128, KC * BL * T], DT, isOutput=True)
        dbg_ho = nc.declare_dram_parameter("dbg_ho", [128, KC * T * BL], DT, isOutput=True)
        dbg_md = nc.declare_dram_parameter("dbg_md", [BL, T], DT, isOutput=True)

    with tile.TileContext(nc) as tc:
        with (
            tc.tile_pool(name="persist", bufs=1) as pp,
            tc.tile_pool(name="work", bufs=6) as wp,
        ):
            # ---- persistent SBUF tiles ----
            u_sb = pp.tile([128, KC, D], DT, tag="u")
            cwt_sb = pp.tile([128, KC, D], DT, tag="cwt")
            ident = pp.tile([128, 128], DT, tag="ident")
            b1_sb = pp.tile([128, KC], DT, tag="b1")
            b2_sb = pp.tile([128, KC], DT, tag="b2")
            bd_sb = pp.tile([128, KC], DT, tag="bd")
            si_sb = pp.tile([T, BL], mybir.dt.int32, tag="si")
            ti_sb = pp.tile([T, BL], mybir.dt.int32, tag="ti")
            madd = pp.tile([BL, T], DT, tag="madd")
            xs = pp.tile([128, KC, BL, T], DT, tag="xs")  # x_src' [d_loc,k,b,t]
            xt = pp.tile([128, KC, BL, T], DT, tag="xt")  # x_tgt'
            hd_all = pp.tile([128, KC, BL, T], DT, tag="hd")  # H' [d_loc,k,b,t]
            ht_all = pp.tile([128, BL, KC, 128], DT, tag="ht")  # H_T [t,b,k,d_loc]
            h1 = pp.tile([128, KC, BL], DT, tag="h1")
            hdec = pp.tile([128, KC, BL], DT, tag="hdec")
            houts = pp.tile([128, KC, T * BL], DT, tag="houts")  # outs' [d,k,t*4+b]

            # ---- load constants ----
            for k in range(KC):
                nc.sync.dma_start(out=u_sb[:, k, :], in_=u_d[k * 128:(k + 1) * 128, :])
                nc.sync.dma_start(out=cwt_sb[:, k, :], in_=cwt_d[k * 128:(k + 1) * 128, :])
            nc.sync.dma_start(out=b1_sb[:, :], in_=b1_d[:, :])
            nc.sync.dma_start(out=b2_sb[:, :], in_=b2_d[:, :])
            nc.sync.dma_start(out=bd_sb[:, :], in_=bd_d[:, :])
            nc.sync.dma_start(out=si_sb[:, :], in_=si_d[:, :])
            nc.sync.dma_start(out=ti_sb[:, :], in_=ti_d[:, :])
            make_identity(nc, ident[:, :])

            # ---- mask: madd[b, t] = (src==0) * -1e9, built as (T,BL) then PE-transposed
            with tc.tile_pool(name="pst", bufs=2, space="PSUM") as pst:
                mf = wp.tile([T, BL], DT, tag="mf")
                nc.vector.tensor_copy(out=mf[:, :], in_=si_sb[:, :])  # int->f32 cast
                m01 = wp.tile([T, BL], DT, tag="m01")
                nc.vector.tensor_scalar(
                    out=m01[:, :], in0=mf[:, :], scalar1=0.0, scalar2=None,
                    op0=ALU.is_equal,
                )
                mps = pst.tile([BL, T], DT, tag="mps")
                nc.tensor.matmul(out=mps[:, :], lhsT=m01[:, :], rhs=ident[:, :],
                                 start=True, stop=True)
                nc.vector.tensor_scalar(
                    out=madd[:, :], in0=mps[:, :], scalar1=-1e9, scalar2=None,
                    op0=ALU.mult,
                )

                # ---- gather embeddings + transpose to [d_loc, k, b, t] ----
                for (idx_sb, e_d, xdst) in ((si_sb, een_d, xs), (ti_sb, ede_d, xt)):
                    for b in range(BL):
                        xg = wp.tile([T, D], DT, tag="xg")
                        nc.gpsimd.indirect_dma_start(
                            out=xg[:, :],
                            out_offset=None,
                            in_=e_d[:, :],
                            in_offset=bass.IndirectOffsetOnAxis(
                                ap=idx_sb[:, b:b + 1], axis=0),
                        )
                        for k in range(KC):
                            tp = pst.tile([128, 128], DT, tag="tp")
                            nc.tensor.matmul(
                                out=tp[:, :], lhsT=xg[:, k * 128:(k + 1) * 128],
                                rhs=ident[:, :], start=True, stop=True)
                            nc.vector.tensor_copy(out=xdst[:, k, b, :], in_=tp[:, :])

            # ---- encoder ----
            with tc.tile_pool(name="pse", bufs=6, space="PSUM") as pse:
                for t in range(T):
                    # layer 1: all matmuls first (they read the OLD h1),
                    # then the adds+tanhs that overwrite h1.
                    if t == 0:
                        for m in range(KC):
                            nc.scalar.activation(
                                out=h1[:, m, :], in_=xs[:, m, :, 0], func=AF.Tanh,
                                bias=b1_sb[:, m:m + 1])
                    else:
                        pls = []
                        for m in range(KC):
                            ps = pse.tile([128, BL], DT, tag="ps")
                            for k in range(KC):
                                nc.tensor.matmul(
                                    out=ps[:, :],
                                    lhsT=u_sb[:, k, m * 128:(m + 1) * 128],
                                    rhs=h1[:, k, :],
                                    start=(k == 0), stop=(k == KC - 1))
                            pls.append(ps)
                        for m in range(KC):
                            tmp = wp.tile([128, BL], DT, tag="tmp")
                            nc.vector.tensor_add(out=tmp[:, :], in0=pls[m][:, :],
                                                 in1=xs[:, m, :, t])
                            nc.scalar.activation(
                                out=h1[:, m, :], in_=tmp[:, :], func=AF.Tanh,
                                bias=b1_sb[:, m:m + 1])
                    # layer 2: reads hd_all[..., t-1] (no alias) + new h1
                    for m in range(KC):
                        if t == 0:
                            nc.scalar.activation(
                                out=hd_all[:, m, :, 0], in_=h1[:, m, :], func=AF.Tanh,
                                bias=b2_sb[:, m:m + 1])
                        else:
                            ps = pse.tile([128, BL], DT, tag="ps")
                            for k in range(KC):
                                nc.tensor.matmul(
                                    out=ps[:, :],
                                    lhsT=u_sb[:, k, m * 128:(m + 1) * 128],
                                    rhs=hd_all[:, k, :, t - 1],
                                    start=(k == 0), stop=(k == KC - 1))
                            tmp = wp.tile([128, BL], DT, tag="tmp")
                            nc.vector.tensor_add(out=tmp[:, :], in0=ps[:, :],
                                                 in1=h1[:, m, :])
                            nc.scalar.activation(
                                out=hd_all[:, m, :, t], in_=tmp[:, :], func=AF.Tanh,
                                bias=b2_sb[:, m:m + 1])

            # ---- H' -> H_T transposes ----
            with tc.tile_pool(name="pst2", bufs=4, space="PSUM") as pst2:
                for b in range(BL):
                    for k in range(KC):
                        tp = pst2.tile([128, 128], DT, tag="tp2")
                        nc.tensor.matmul(out=tp[:, :], lhsT=hd_all[:, k, b, :],
                                         rhs=ident[:, :], start=True, stop=True)
                        nc.vector.tensor_copy(out=ht_all[:, b, k, :], in_=tp[:, :])

            # ---- decoder ----
            with (
                tc.tile_pool(name="ps_h", bufs=2, space="PSUM") as ps_h,
                tc.tile_pool(name="ps_a", bufs=1, space="PSUM") as ps_a,
                tc.tile_pool(name="ps_c", bufs=2, space="PSUM") as ps_c,
                tc.tile_pool(name="psS", bufs=2, space="PSUM") as psS,
            ):
                for t in range(T):
                    # h = tanh(h@U + x_t + b): matmuls first (read OLD hdec)
                    phs = []
                    for m in range(KC):
                        ps = ps_h.tile([128, BL], DT, tag="ph")
                        for k in range(KC):
                            prev = hd_all[:, k, :, T - 1] if t == 0 else hdec[:, k, :]
                            nc.tensor.matmul(
                                out=ps[:, :],
                                lhsT=u_sb[:, k, m * 128:(m + 1) * 128],
                                rhs=prev,
                                start=(k == 0), stop=(k == KC - 1))
                        phs.append(ps)
                    for m in range(KC):
                        tmp = wp.tile([128, BL], DT, tag="tmp")
                        nc.vector.tensor_add(out=tmp[:, :], in0=phs[m][:, :],
                                             in1=xt[:, m, :, t])
                        nc.scalar.activation(
                            out=hdec[:, m, :], in_=tmp[:, :], func=AF.Tanh,
                            bias=bd_sb[:, m:m + 1])
                    # scores, transposed: S_T[t, b] = H_b'[:, t] . h_b
                    stp = psS.tile([128, BL], DT, tag="sps")
                    for b in range(BL):
                        for k in range(KC):
                            nc.tensor.matmul(
                                out=stp[:, b:b + 1], lhsT=hd_all[:, k, b, :],
                                rhs=hdec[:, k, b:b + 1],
                                start=(k == 0), stop=(k == KC - 1))
                    st_sb = wp.tile([128, BL], DT, tag="st_sb")
                    nc.vector.tensor_copy(out=st_sb[:, :], in_=stp[:, :])
                    # transpose to (BL, T) for the softmax
                    sps2 = ps_a.tile([BL, T], DT, tag="sps2")
                    nc.tensor.matmul(out=sps2[:, :], lhsT=st_sb[:, :],
                                     rhs=ident[:, :], start=True, stop=True)
                    # masked softmax over t with scale 1/16 folded into exp
                    s_sb = wp.tile([BL, T], DT, tag="s_sb")
                    nc.vector.tensor_add(out=s_sb[:, :], in0=sps2[:, :],
                                         in1=madd[:, :])
                    mx = wp.tile([BL, 1], DT, tag="mx")
                    nc.vector.reduce_max(out=mx[:, :], in_=s_sb[:, :], axis=AX.X)
                    nc.vector.tensor_scalar(
                        out=s_sb[:, :], in0=s_sb[:, :], scalar1=mx[:, :1],
                        scalar2=None, op0=ALU.subtract)
                    ex = wp.tile([BL, T], DT, tag="ex")
                    nc.scalar.activation(out=ex[:, :], in_=s_sb[:, :], func=AF.Exp,
                                         scale=1.0 / 16.0)
                    sm = wp.tile([BL, 1], DT, tag="sm")
                    nc.vector.reduce_sum(out=sm[:, :], in_=ex[:, :], axis=AX.X)
                    rs = wp.tile([BL, 1], DT, tag="rs")
                    nc.vector.reciprocal(out=rs[:, :], in_=sm[:, :])
                    alpha = wp.tile([BL, T], DT, tag="alpha")
                    nc.vector.tensor_scalar(
                        out=alpha[:, :], in0=ex[:, :], scalar1=rs[:, :1],
                        scalar2=None, op0=ALU.mult)
                    # alpha (BL,T) -> alphaT (T,BL)
                    aps = ps_a.tile([128, BL], DT, tag="aps")
                    nc.tensor.matmul(out=aps[:, :], lhsT=alpha[:, :],
                                     rhs=ident[:BL, :BL], start=True, stop=True)
                    a_t = wp.tile([128, BL], DT, tag="a_t")
                    nc.vector.tensor_copy(out=a_t[:, :], in_=aps[:, :])
                    # ctx'[d_chunk m, b] = sum_t H_T[t,b,m,:]^T @ alphaT[:,b]
                    ctxs = wp.tile([128, KC, BL], DT, tag="ctxs")
                    for m in range(KC):
                        cps = ps_c.tile([128, BL], DT, tag="cps")
                        for b in range(BL):
                            nc.tensor.matmul(
                                out=cps[:, b:b + 1], lhsT=ht_all[:, b, m, :],
                                rhs=a_t[:, b:b + 1], start=True, stop=True)
                        nc.vector.tensor_copy(out=ctxs[:, m, :], in_=cps[:, :])
                    # out' = h' + ctx_W @ ctx'   -> houts[:, m, t*BL:(t+1)*BL]
                    for m in range(KC):
                        ops_ = ps_h.tile([128, BL], DT, tag="ph")
                        for k in range(KC):
                            nc.tensor.matmul(
                                out=ops_[:, :],
                                lhsT=cwt_sb[:, k, m * 128:(m + 1) * 128],
                                rhs=ctxs[:, k, :],
                                start=(k == 0), stop=(k == KC - 1))
                        nc.vector.tensor_add(
                            out=houts[:, m, t * BL:(t + 1) * BL],
                            in0=ops_[:, :], in1=hdec[:, m, :])

            # ---- final projection: logits = outs @ W_out.T ----
            if DEBUG:
                nc.sync.dma_start(out=dbg_xs[:, :], in_=xs[:, :, :, :])
                nc.sync.dma_start(out=dbg_hd[:, :], in_=hd_all[:, :, :, :])
                nc.sync.dma_start(out=dbg_ho[:, :], in_=houts[:, :, :])
                nc.sync.dma_start(out=dbg_md[:, :], in_=madd[:, :])
            n_sizes = []
            n0 = 0
            while n0 < V:
                n_sizes.append((n0, min(512, V - n0)))
                n0 += 512
            with (
                tc.tile_pool(name="psL", bufs=4, space="PSUM") as psL,
                tc.tile_pool(name="wpool", bufs=4) as wpool,
                tc.tile_pool(name="lpool", bufs=4) as lpool,
            ):
                for (n0, nv) in n_sizes:
                    wt = []
                    for k in range(KC):
                        wk = wpool.tile([128, 512], DT, tag="wk")
                        nc.sync.dma_start(out=wk[:, :nv],
                                          in_=wot_d[k * 128:(k + 1) * 128, n0:n0 + nv])
                        wt.append(wk)
                    for mt in range(T * BL // 128):
                        pl = psL.tile([128, 512], DT, tag="pl")
                        for k in range(KC):
                            nc.tensor.matmul(
                                out=pl[:, :nv],
                                lhsT=houts[:, k, mt * 128:(mt + 1) * 128],
                                rhs=wt[k][:, :nv],
                                start=(k == 0), stop=(k == KC - 1))
                        lt = lpool.tile([128, 512], DT, tag="lt")
                        nc.vector.tensor_copy(out=lt[:, :nv], in_=pl[:, :nv])
                        nc.sync.dma_start(
                            out=out_d[mt * 128:(mt + 1) * 128, n0:n0 + nv],
                            in_=lt[:, :nv])
    nc.compile()
    return nc


def _prep_in_maps(U, b_enc1, b_enc2, b_dec, E_en, E_de, ctx_W, W_out_de,
                  src_en, tgt_de_in):
    f32 = np.float32
    U = np.ascontiguousarray(U, f32)
    ctx_wt = np.ascontiguousarray(np.asarray(ctx_W, f32).T)
    w_out_t = np.ascontiguousarray(np.asarray(W_out_de, f32).T)
    E_en = np.ascontiguousarray(E_en, f32)
    E_de = np.ascontiguousarray(E_de, f32)
    b1 = np.ascontiguousarray(np.asarray(b_enc1, f32).reshape(KC, 128).T)
    b2 = np.ascontiguousarray(np.asarray(b_enc2, f32).reshape(KC, 128).T)
    bd = np.ascontiguousarray(np.asarray(b_dec, f32).reshape(KC, 128).T)
    src = np.asarray(src_en).astype(np.int32)
    tgt = np.asarray(tgt_de_in).astype(np.int32)
    in_maps = []
    for i in range(NCORES):
        b0 = i * BL
        in_maps.append({
            "u": U, "ctx_wt": ctx_wt, "w_out_t": w_out_t,
            "e_en": E_en, "e_de": E_de,
            "b1": b1, "b2": b2, "bd": bd,
            "src_idx": np.ascontiguousarray(src[:, b0:b0 + BL]),
            "tgt_idx": np.ascontiguousarray(tgt[:, b0:b0 + BL]),
        })
    return in_maps


def kernel(U, b_enc1, b_enc2, b_dec, E_en, E_de, ctx_W, W_out_de,
           src_en, tgt_de_in, _trace=False, _raw=False):
    if "nc" not in _CACHE:
        _CACHE["nc"] = _build()
    nc = _CACHE["nc"]
    in_maps = _prep_in_maps(U, b_enc1, b_enc2, b_dec, E_en, E_de, ctx_W,
                            W_out_de, src_en, tgt_de_in)
    res = run_bass_kernel_spmd(nc, in_maps, list(range(NCORES)), trace=_trace)
    if _raw:
        return res
    logits = np.empty((T, B, V), np.float32)
    for i in range(NCORES):
        logits[:, i * BL:(i + 1) * BL, :] = res.results[i]["out"].reshape(T, BL, V)
    if _trace:
        return logits, res
    return logits
